# revision 47
# baseline (speedup 1.0000x reference)
"""Trainium2 Bass kernel for nn_Block_47502338294589 (dense transformer block).

Block (B=4, T=1024, C=1024, H=16 heads, D=64):
    x += causal_selfattn(LN1(x)) @ attn_proj
    x += crossattn(x, visual_features) @ ca_proj
    x += MLP(LN2(x))          (tanh GELU, 4C hidden)
    x += adapter(x)           (exact GELU, 256 hidden)

Host-side algebra (pure functions of the inputs):
  - cross-attention keys/values are identical at every position, so its
    softmax is uniform and the whole branch collapses to a per-batch
    additive vector, folded into the attn-proj residual bias;
  - LN gains fold into the consuming weights; LN1 itself is computed on
    the host and shipped pre-transposed (fp8) so the device's first
    matmul starts as soon as the first DMA chunk lands;
  - every weight matrix is pre-rearranged so each device DMA reads
    fully contiguous memory (sub-512B runs pay a 2x DMA penalty).

Precision: residual stream / LN stats / softmax sums fp32; S and O
matmuls + MLP in fp16 (same PE rate as bf16, 8x less rounding error);
q/k/v projections, attn-proj and ad_up in fp8e4 with
perf_mode=DoubleRow (2 contraction tiles per matmul -> ~2x PE rate).
fp8 weights are pre-scaled x32 on the host (dodges e4m3 subnormals at
|w|~0.02); the descale is free: folded into the softmax exp scale
(1/32^2) and the PSUM-eviction scalar multipliers.

Sharding: sequence-parallel, 8 cores = 4 batches x 2 sequence halves,
no collectives. Core c computes the 512 query rows [512*(c%2), ...) of
batch c//2, with keys [A | B]: A = rows 0:512 masked per-core via a
log-bias input, B = own rows with compile-time causal structure.

Device schedule:
  - attention is ACT(exp)-throughput-bound relative to its own matmuls,
    so q/k projections for head-pair hp+1 are emitted interleaved
    between the S and O matmuls of head-pair hp (software pipeline);
    V is computed for all heads upfront in N=512 DoubleRow matmuls;
  - all weights stream on the gpsimd SWDGE queue (separate pipe from
    the HWDGE used by activations/transposes);
  - LN2 row-sums ride the proj evictions' accum_out; sum(x^2) is one
    ACT Square pass; rsqrt runs as exp(-0.5*ln(var+eps)) and the
    normalize as ACT Identity(x*rstd - mean*rstd), so ACT stays in the
    natural_log_exp table set and the DVE only does [P,1] algebra; the
    adapter uses tanh-GELU so only one table switch happens (exp->gelu);
  - fc/mproj run half fp8-DoubleRow, half fp16 (k-split): full-fp8 MLP
    would blow the 2e-2 error gate; the early causal rows additionally
    get hi/lo-corrected V and an fp16 m=0 attn-proj (their attention
    output barely averages, so fp8 noise there dominated the absmax);
  - the feature-major transposes needed by fc/adapter run on the SP
    HWDGE queue, off the ACT datapath.

Self-contained: hardcodes shapes; needs numpy/ml_dtypes + concourse.
"""

import numpy as np
import ml_dtypes

B, T, C, H, D = 4, 1024, 1024, 16, 64
TQ = 512            # query rows per core
TKV = 1024          # A (512) + B (512) key rows per core
FF = 4 * C
DOWN = 256
P = 128
NCORES = 8
NEG = -30000.0      # exp(x + NEG) == 0 in fp32
WS = 32.0           # fp8 weight pre-scale
EXPS = 0.125 / (WS * WS)   # softmax scale incl. q/k fp8 descale
J8 = 4              # fc k-tiles (of 8) in fp8   (fraction of C)
K8 = 16             # mproj k-tiles (of 32) in fp8 (fraction of 4C)

_CACHE = {}


# --------------------------------------------------------------------------
# walrus workaround: setupSyncWait accepts at most 2 sync-wait commands per
# instruction (and lowering may add one of its own), while Tile's semaphore
# pass can attach more. Hoist excess waits onto same-engine NoOps placed
# immediately before the offending instruction; in-order execution keeps
# the semantics identical.
def _split_excess_waits(nc, max_waits=1):
    import concourse.mybir as mybir
    n_new = 0
    for fn in nc.m.functions:
        for bb in fn.blocks:
            out, changed = [], False
            for ins in bb.instructions:
                si = ins.sync_info
                if si is not None and si.on_wait is not None \
                        and len(si.on_wait) > max_waits:
                    waits = list(si.on_wait)
                    extra, keep = waits[:-max_waits], waits[-max_waits:]
                    for j in range(0, len(extra), max_waits):
                        n_new += 1
                        out.append(mybir.InstNoOp(
                            name=f"I-waitsplit-{n_new}",
                            engine=ins.engine,
                            bass_nofuse=True,
                            sync_info=mybir.SyncInfo(
                                on_wait=extra[j:j + max_waits], on_update=[]),
                        ))
                    si.on_wait = keep
                    ins.sync_info = si
                    changed = True
                out.append(ins)
            if changed:
                bb.instructions = out
    return n_new


def _build_program():
    import concourse.bass as bass
    import concourse.mybir as mybir
    from concourse.tile import TileContext

    dt = mybir.dt
    f32, f16, f8 = dt.float32, dt.float16, dt.float8e4
    AF = mybir.ActivationFunctionType
    ALU = mybir.AluOpType
    DR = mybir.MatmulPerfMode.DoubleRow

    nc = bass.Bass()

    def din(name, shape, dtype=f32):
        return nc.dram_tensor(name, shape, dtype, kind="ExternalInput")

    x_q = din("x_q", [TQ, C])            # query rows, ca/proj bias pre-added
    # hT_in[p, a, e, i, f] = LN1(x_kv)[i*128+f, (2a+e)*128+p] in fp8 --
    # feature-pair-major so DoubleRow matmuls slice [p, 2, N] 3D APs.
    hT_in = din("hT_in", [P, 4, 2, 8, P], f8)
    # x32 fp8 residual of hT tile i=4 (this core's first 128 own rows):
    # feeds the hi/lo-corrected V projection for the early causal rows,
    # whose attention output barely averages and is fp8-sensitive.
    h_lo4 = din("h_lo4", [P, 4, 2, P], f8)
    log_s = din("log_s", [1, 1])         # 0.0 (A visible) or NEG (A masked)
    # weights, host-prearranged for contiguous DMA (see _prep_inputs)
    # per-head-pair fused q/k weights: cols = [q 128 | k 128], x32 fp8
    w_qkv = din("w_qkv", [8, P, 4, 2, 2 * P], f8)  # [hp, p, a, e, 256]
    w_v = din("w_v", [2, P, 4, 2, TQ], f8)         # [n2, p, a, e, c] x32
    w_v_lo = din("w_v_lo", [2, P, 4, 2, TQ], f8)   # x1024 fp8 residual
    w_pj = din("w_pj", [P, 4, 2, C], f8)           # [p, a, e, c] x32
    w_pj16 = din("w_pj16", [P, 8, C], f16)         # fp16 copy (m=0 proj)
    w_fc8 = din("w_fc8", [16, P, 2, 2, 2 * P], f8)   # k 0:J8 x32 fp8 pairs
    w_fc16 = din("w_fc16", [16, P, 8 - J8, 2 * P], f16)  # k J8:8 x32
    w_mp8 = din("w_mp8", [P, K8 // 2, 2, C], f8)   # k 0:K8 x32 fp8 pairs
    w_mp16 = din("w_mp16", [P, 32 - K8, C], f16)   # k K8:32 x32
    w_ad = din("w_ad", [P, 8, DOWN], f16)          # [p, k, c]
    w_au = din("w_au", [P, 2, C], f16)             # [p, k, c]
    bqk_T = din("bqk_T", [P, 16])        # attn_b[:2C] partition-major (x32)
    fcb_T = din("fcb_T", [P, FF // P])   # fc_b partition-major
    adb_T = din("adb_T", [P, DOWN // P])  # ad_down_b partition-major
    tri = din("tri", [P, P], f16)        # tri[k, q] = 1 if k <= q
    y_out = nc.dram_tensor("y", [TQ, C], f32, kind="ExternalOutput")

    x_q_r = x_q.rearrange("(i p) c -> i p c", p=P)
    y_r = y_out.rearrange("(i p) c -> i p c", p=P)

    with TileContext(nc) as tc:
        with tc.tile_pool(name="res", bufs=1) as res, \
             tc.tile_pool(name="scr", bufs=3) as scr, \
             tc.tile_pool(name="wfcp", bufs=12) as wfcp, \
             tc.tile_pool(name="wf8p", bufs=16) as wf8p, \
             tc.tile_pool(name="wmpp", bufs=4) as wmpp:

            # hT arrives pre-normalized/pre-transposed fp8 from the host;
            # DMAs head the SP queue, B tiles (i=4:8) per k-pair first so
            # the prologue V/q/k matmuls start as chunks land.
            hT = res.tile([P, 4, 2, 8, P], f8, tag="hT", name="hT")
            for a in range(4):
                nc.sync.dma_start(hT[:, a, :, 4:8, :], hT_in[:, a, :, 4:8, :])
            nc.sync.dma_start(hT[:, 0:2, :, 0:4, :], hT_in[:, 0:2, :, 0:4, :])
            nc.sync.dma_start(hT[:, 2:4, :, 0:4, :], hT_in[:, 2:4, :, 0:4, :])

            # ---- constants -------------------------------------------------
            logs_b = res.tile([P, 1], f32, tag="logs", name="logs")
            nc.sync.dma_start(logs_b[:], log_s[:].to_broadcast((P, 1)))
            bqk_sb = res.tile([P, 16], f32, tag="bqk", name="bqk")
            nc.sync.dma_start(bqk_sb[:], bqk_T[:])
            fcb_sb = res.tile([P, FF // P], f32, tag="fcb", name="fcb")
            nc.sync.dma_start(fcb_sb[:], fcb_T[:])
            adb_sb = res.tile([P, DOWN // P], f32, tag="adb", name="adb")
            nc.sync.dma_start(adb_sb[:], adb_T[:])
            tri_sb = res.tile([P, P], f16, tag="tri", name="tri")
            nc.sync.dma_start(tri_sb[:], tri[:])
            ones_sb = res.tile([1, 64], f16, tag="ones", name="ones")
            nc.vector.memset(ones_sb[:], 1.0)
            eps_sb = res.tile([P, 1], f32, tag="eps", name="eps")
            nc.vector.memset(eps_sb[:], 1e-5)

            x1 = [res.tile([P, C], f32, tag=f"x1_{m}", name=f"x1_{m}") for m in range(4)]
            ln2h = [res.tile([P, C], f16, tag=f"l2h{m}", name=f"l2h{m}")
                    for m in range(4)]
            # ln2T_all[p, m, j, f] = ln2h[m][f, j*128+p]  (batched transpose out)
            ln2T = res.tile([P, 4, 8, P], f16, tag="l2T", name="l2T")
            # fp8 copy of ln2T's first J8 feature tiles, j-major so a
            # DoubleRow pair slice is a 3D [p, 2, 512] AP (j stride 512).
            ln2T8 = res.tile([P, J8, 4, P], f8, tag="l2T8", name="l2T8")

            def layernorm_finish(x_ap, out_16, sx, sq):
                """out = (x - mean)*rsqrt(var+eps) given sx[P,2] = per-half
                row sums of x (from the eviction STTs' accum_out) and
                sq[P,1] = row sum of x^2 (one full-width ACT Square).
                rsqrt = exp(-0.5*ln(var+eps)) keeps ACT in the exp/ln
                table set; the [P,C] normalize runs on ACT as
                Identity(x*rstd + (-mean*rstd)) -- the DVE only does tiny
                [P,1] algebra."""
                t = scr.tile([P, 2], f32, tag="ln_t", name="ln_t")
                nc.vector.tensor_tensor(t[:, 0:1], sx[:, 0:1], sx[:, 1:2],
                                        ALU.add)
                nc.vector.tensor_tensor(t[:, 1:2], t[:, 0:1], t[:, 0:1],
                                        ALU.mult)
                u = scr.tile([P, 1], f32, tag="ln_u", name="ln_u")
                # u = sum(x^2) - (sum x)^2/C ; var = u/C
                nc.vector.scalar_tensor_tensor(
                    out=u[:], in0=t[:, 1:2], scalar=-1.0 / C, in1=sq[:],
                    op0=ALU.mult, op1=ALU.add)
                lnv = scr.tile([P, 1], f32, tag="ln_lnv", name="ln_lnv")
                nc.scalar.activation(lnv[:], u[:], AF.Ln,
                                     bias=eps_sb[:], scale=1.0 / C)
                rstd = scr.tile([P, 1], f32, tag="ln_rstd", name="ln_rstd")
                nc.scalar.activation(rstd[:], lnv[:], AF.Exp, scale=-0.5)
                nb = scr.tile([P, 1], f32, tag="ln_nb", name="ln_nb")
                nc.vector.scalar_tensor_tensor(
                    out=nb[:], in0=t[:, 0:1], scalar=-1.0 / C, in1=rstd[:],
                    op0=ALU.mult, op1=ALU.mult)
                nc.scalar.activation(out_16, x_ap, AF.Identity,
                                     bias=nb[:], scale=rstd[:])

            # =========== phase A: LN1, fused qkv+attention, attn-proj ======
            # Attention is ACT(exp)-bound relative to its own matmuls, so
            # qkv production is fused into the per-head-pair loop: PE
            # computes hp's q/k/v and S/O while ACT streams previous exps.
            with tc.tile_pool(name="pA", bufs=1) as pA, \
                 tc.tile_pool(name="wqkp", bufs=3) as wqkp, \
                 tc.tile_pool(name="wvp", bufs=2) as wvp, \
                 tc.tile_pool(name="hpp", bufs=2) as hpp, \
                 tc.tile_pool(name="psQ", bufs=1, space="PSUM") as psQ, \
                 tc.tile_pool(name="psK", bufs=2, space="PSUM") as psK, \
                 tc.tile_pool(name="psS", bufs=3, space="PSUM") as psS, \
                 tc.tile_pool(name="psO", bufs=2, space="PSUM") as psO:
                # V shares psS's three banks (its phase precedes the S/O
                # loop); psS bufs=3 lets S(kt+1) run ahead of exp(kt) so
                # the ACT exp stream -- the attention-loop bottleneck --
                # never starves.
                psV = psS
                oT = pA.tile([P, 4, 2, TQ], f8, tag="oT", name="oT")
                # fp16 copy of oT's first 128 token columns: proj for the
                # early causal rows runs in fp16 (fp8 o there is the
                # dominant error source -- little softmax averaging).
                oT16 = pA.tile([P, 8, P], f16, tag="oT16", name="oT16")
                v_sb = pA.tile([P, 8, H, 65], f16, tag="vsb", name="vsb")
                w_pj_sb = pA.tile([P, 4, 2, C], f8, tag="wpj", name="wpj")
                w_pj16_sb = pA.tile([P, 8, C], f16, tag="wpj16", name="wpj16")
                h_lo_sb = pA.tile([P, 4, 2, P], f8, tag="hlo", name="hlo")
                nc.sync.dma_start(h_lo_sb[:], h_lo4[:])

                # --- software-pipelined per-head-pair qkv + attention ---
                # DoubleRow q/k/v: contraction pair a covers feature tiles
                # (2a, 2a+1); four a-steps replace eight k-steps.
                def alloc_hp(hp):
                    t = {}
                    t["wch"] = wqkp.tile([P, 4, 2, 2 * P], f8, tag="wqkv", name="wqkv")
                    nc.gpsimd.dma_start(t["wch"][:, 0:2, :, :], w_qkv[hp, :, 0:2, :, :])
                    nc.gpsimd.dma_start(t["wch"][:, 2:4, :, :], w_qkv[hp, :, 2:4, :, :])
                    t["qT"] = hpp.tile([P, TQ], f16, tag="qT", name="qT")
                    t["kT"] = hpp.tile([P, TKV], f16, tag="kT", name="kT")
                    t["pq"] = psQ.tile([P, TQ], f32, tag="q", name="q")
                    t["pk0"] = psK.tile([P, TQ], f32, tag="k", name="k")
                    t["pk1"] = psK.tile([P, TQ], f32, tag="k", name="k")
                    return t

                def emit_qk(hp, t, a, which):
                    st, sp = (a == 0), (a == 3)
                    wch = t["wch"]
                    if which == "q":
                        nc.tensor.matmul(t["pq"][:], wch[:, a, :, 0:P],
                                         hT[:, a, :, 4:8, :], start=st, stop=sp,
                                         perf_mode=DR)
                        if sp:
                            nc.vector.tensor_scalar_add(
                                t["qT"][:], t["pq"][:], bqk_sb[:, hp:hp + 1])
                    elif which == "k0":
                        nc.tensor.matmul(t["pk0"][:], wch[:, a, :, P:2 * P],
                                         hT[:, a, :, 0:4, :], start=st, stop=sp,
                                         perf_mode=DR)
                        if sp:
                            nc.vector.tensor_scalar_add(
                                t["kT"][:, 0:TQ], t["pk0"][:],
                                bqk_sb[:, 8 + hp:9 + hp])
                    else:
                        nc.tensor.matmul(t["pk1"][:], wch[:, a, :, P:2 * P],
                                         hT[:, a, :, 4:8, :], start=st, stop=sp,
                                         perf_mode=DR)
                        if sp:
                            nc.vector.tensor_scalar_add(
                                t["kT"][:, TQ:TKV], t["pk1"][:],
                                bqk_sb[:, 8 + hp:9 + hp])

                def emit_qkv_step(hp, t, a):
                    emit_qk(hp, t, a, "q")
                    emit_qk(hp, t, a, "k0")
                    emit_qk(hp, t, a, "k1")

                # prologue: hp0's q starts as soon as the B-chunks of hT
                # land (before V -- it only needs the same chunks), then
                # batched V for ALL head pairs in N=512 DoubleRow matmuls.
                # m=4 (this core's first 128 own rows) gets an extra hi/lo
                # correction group after the main sweep: h8@w_lo (x1024) +
                # h_lo@w8 (x1024), folded into v_sb with a x1/32 add --
                # fp16-grade v for the keys the early causal rows attend
                # to. V psums share the psS pool (tag "S").
                nc.vector.memset(v_sb[:, :, :, 64:65], 1.0)
                cur0 = alloc_hp(0)
                for a in range(4):
                    emit_qk(0, cur0, a, "q")
                wvns = []
                for n2 in range(2):
                    wvn = wvp.tile([P, 4, 2, TQ], f8, tag="wvn", name="wvn")
                    nc.gpsimd.dma_start(wvn[:, 0:2, :, :], w_v[n2, :, 0:2, :, :])
                    nc.gpsimd.dma_start(wvn[:, 2:4, :, :], w_v[n2, :, 2:4, :, :])
                    wvns.append(wvn)
                for n2 in range(2):
                    wvn = wvns[n2]
                    for m in (4, 5, 6, 7, 0, 1, 2, 3):
                        pv = psV.tile([P, TQ], f32, tag="S", name="pv")
                        for a in range(4):
                            nc.tensor.matmul(
                                pv[:], hT[:, a, :, m, :], wvn[:, a, :, :],
                                start=(a == 0), stop=(a == 3), perf_mode=DR)
                        # ACT is idle during the V phase; evicting there
                        # frees the shared psS slots at PE rate instead of
                        # queueing behind DVE.
                        nc.scalar.activation(
                            v_sb[:, m, 8 * n2:8 * (n2 + 1), 0:64],
                            pv[:].rearrange("p (h d) -> p h d", d=64),
                            AF.Copy)

                # prologue k for hp=0
                cur = cur0
                for a in range(4):
                    emit_qk(0, cur, a, "k1")
                for a in range(4):
                    emit_qk(0, cur, a, "k0")

                # m=4 hi/lo correction (v_sb[4] is first consumed at kt=4
                # of hp0, ~10us after this)
                wvlos = []
                for n2 in range(2):
                    wvlo = wvp.tile([P, 4, 2, TQ], f8, tag="wvlo", name="wvlo")
                    nc.gpsimd.dma_start(wvlo[:], w_v_lo[n2])
                    wvlos.append(wvlo)
                for n2 in range(2):
                    pvc = psV.tile([P, TQ], f32, tag="S", name="pvc")
                    for a in range(4):
                        nc.tensor.matmul(
                            pvc[:], hT[:, a, :, 4, :], wvlos[n2][:, a, :, :],
                            start=(a == 0), stop=False, perf_mode=DR)
                    for a in range(4):
                        nc.tensor.matmul(
                            pvc[:], h_lo_sb[:, a, :, :], wvns[n2][:, a, :, :],
                            start=False, stop=(a == 3), perf_mode=DR)
                    nc.vector.scalar_tensor_tensor(
                        out=v_sb[:, 4, 8 * n2:8 * (n2 + 1), 0:64],
                        in0=pvc[:].rearrange("p (h d) -> p h d", d=64),
                        scalar=1.0 / WS,
                        in1=v_sb[:, 4, 8 * n2:8 * (n2 + 1), 0:64],
                        op0=ALU.mult, op1=ALU.add)
                with tc.tile_wait_until(0.03):
                    nc.gpsimd.dma_start(w_pj_sb[:], w_pj[:])
                    nc.gpsimd.dma_start(w_pj16_sb[:], w_pj16[:])
                    for m in range(4):
                        nc.gpsimd.dma_start(x1[m][:], x_q_r[m])

                def emit_proj_m(m, a_lo, a_hi, sx=None):
                    """proj partial over hp-pair range [a_lo, a_hi) for
                    token tile m, evict-added into x1. accum_out (row
                    sums for LN2) only on the final partial, whose STT
                    output is the completed x1."""
                    for n2 in range(2):
                        pool, ptag = (psS, "S") if n2 == 0 else (psO, "O")
                        pt = pool.tile([P, TQ], f32, tag=ptag, name="Spj")
                        if m == 0:
                            for k in range(2 * a_lo, 2 * a_hi):
                                nc.tensor.matmul(
                                    pt[:, 0:TQ], oT16[:, k, :],
                                    w_pj16_sb[:, k, TQ * n2:TQ * (n2 + 1)],
                                    start=(k == 2 * a_lo), stop=(k == 2 * a_hi - 1))
                        else:
                            for a in range(a_lo, a_hi):
                                nc.tensor.matmul(
                                    pt[:], oT[:, a, :, P * m:P * (m + 1)],
                                    w_pj_sb[:, a, :, TQ * n2:TQ * (n2 + 1)],
                                    start=(a == a_lo), stop=(a == a_hi - 1),
                                    perf_mode=DR)
                        nc.vector.scalar_tensor_tensor(
                            out=x1[m][:, TQ * n2:TQ * (n2 + 1)], in0=pt[:],
                            scalar=(1.0 / WS if m == 0 else 1.0 / (WS * WS)),
                            in1=x1[m][:, TQ * n2:TQ * (n2 + 1)],
                            op0=ALU.mult, op1=ALU.add,
                            accum_out=(sx[:, n2:n2 + 1] if sx is not None
                                       else None))
                    return sx

                for hp in range(8):
                    nxt = alloc_hp(hp + 1) if hp + 1 < 8 else None
                    qT, kT = cur["qT"], cur["kT"]
                    pO = [psO.tile([65, TQ], f32, tag="O", name="O")
                          for _ in range(2)]
                    for kt in range(8):
                        is_b = kt >= 4
                        q0 = P * (kt - 4) if is_b else 0
                        nq = TQ - q0
                        ksl = slice(P * kt, P * (kt + 1))
                        pS = [None, None]
                        for hh in range(2):
                            rows = slice(64 * hh, 64 * (hh + 1))
                            pS[hh] = psS.tile([P, TQ], f32, tag="S", name="S")
                            nc.tensor.matmul(
                                pS[hh][:, 0:nq], kT[rows, ksl],
                                qT[rows, q0:TQ], start=True, stop=True)
                        # next hp's q/k in the FIRST four kts: their DVE
                        # evictions then precede this hp's oT-normalize
                        # chain in the in-order DVE queue, so the next
                        # hp's S (and the ACT exp stream) start without
                        # the ~3us eviction-behind-normalize stall.
                        if nxt is not None and kt < 4:
                            emit_qkv_step(hp + 1, nxt, kt)
                        pT = scr.tile([P, 2, TQ], f16, tag="pT", name="pT")
                        for hh in range(2):
                            if is_b:
                                nc.scalar.activation(
                                    pT[:, hh, 0:nq], pS[hh][:, 0:nq],
                                    AF.Exp, scale=EXPS)
                                nc.vector.tensor_mul(
                                    pT[:, hh, 0:P], pT[:, hh, 0:P], tri_sb[:])
                            else:
                                nc.scalar.activation(
                                    pT[:, hh, 0:nq], pS[hh][:, 0:nq],
                                    AF.Exp, scale=EXPS, bias=logs_b[:])
                        for hh in range(2):
                            nc.tensor.matmul(
                                pO[hh][:, q0:TQ],
                                v_sb[:, kt, 2 * hp + hh, :], pT[:, hh, 0:nq],
                                start=(kt == 0), stop=(kt == 7),
                                skip_group_check=True)
                    for hh in range(2):
                        sums = scr.tile([1, TQ], f16, tag="sums", name="sums",
                                        bufs=2)
                        nc.vector.tensor_copy(sums[:], pO[hh][64:65, :])
                        # pR lives in the psK rotation: in psS it would
                        # hold an S slot hostage until the DVE reciprocal
                        # drains it (stalling the next hp's S->exp
                        # stream); in psO it deadlocks against its own pO
                        # source. psK's tiles are evicted well before the
                        # hp ends, and the displaced wait lands on the
                        # hp+2 q/k stream, which has a whole hp of slack.
                        pR = psK.tile([P, TQ], f32, tag="k", name="pR")
                        nc.tensor.matmul(pR[0:64, :], ones_sb[:], sums[:],
                                         start=True, stop=True)
                        rbc = scr.tile([64, TQ], f16, tag="rbc", name="rbc", bufs=2)
                        with nc.allow_low_precision(reason="1/sum in fp16; "
                                                    "sums are O(1e3), fine"):
                            nc.vector.reciprocal(rbc[:], pR[0:64, :])
                        nc.vector.tensor_mul(
                            oT[64 * hh:64 * (hh + 1), hp // 2, hp % 2, :],
                            pO[hh][0:64, :], rbc[:])
                        nc.vector.tensor_mul(
                            oT16[64 * hh:64 * (hh + 1), hp, :],
                            pO[hh][0:64, 0:P], rbc[:, 0:P])
                    cur = nxt

                # attn projection + residual into x1 (x_q has the collapsed
                # cross-attention + proj biases pre-added on the host).
                # m=0 (the early causal rows) runs fp16 from oT16; m>=1
                # runs fp8 DoubleRow with descale 1/1024 in the eviction.
                # LN2(m) is inline; proj n2=1 uses the psO pool so two
                # m-iterations of evictions can be in flight while the
                # DVE works through the LN chain.
                def emit_ln_m(m, sx):
                    sq = scr.tile([P, 1], f32, tag="ln_sq", name="ln_sq")
                    # discarded payload; only accum_out matters
                    sqd = scr.tile([P, C], f16, tag="ln_sqd", name="ln_sqd",
                                   bufs=2)
                    nc.scalar.activation(sqd[:], x1[m][:], AF.Square,
                                         accum_out=sq[:])
                    layernorm_finish(x1[m][:], ln2h[m][:], sx, sq)
                    nc.sync.dma_start_transpose(ln2T[:, m, :, :], ln2h[m][:])
                    nc.vector.tensor_copy(ln2T8[:, :, m, :],
                                          ln2T[:, m, 0:J8, :])

                # finals only (hp-pair a=3): a=0..3's partials were
                # emitted inside the hp loop where PE/DVE had slack.
                # Emission order = engine-queue order: m0/m1's LN chains
                # go ahead of m2/m3's bulk work so fc's first token-half
                # sweep (which waits on exactly the m0/m1 transposes)
                # isn't head-of-line-blocked behind m2/m3 Square passes.
                sx0 = scr.tile([P, 2], f32, tag="ln_sx", name="ln_sx", bufs=4)
                emit_proj_m(0, 0, 4, sx0)
                emit_ln_m(0, sx0)
                sx1 = scr.tile([P, 2], f32, tag="ln_sx", name="ln_sx", bufs=4)
                emit_proj_m(1, 0, 4, sx1)
                emit_ln_m(1, sx1)
                sx2 = scr.tile([P, 2], f32, tag="ln_sx", name="ln_sx", bufs=4)
                sx3 = scr.tile([P, 2], f32, tag="ln_sx", name="ln_sx", bufs=4)
                emit_proj_m(2, 0, 4, sx2)
                emit_proj_m(3, 0, 4, sx3)
                emit_ln_m(2, sx2)
                emit_ln_m(3, sx3)

            # =========== phase B: LN2, MLP, adapter ========================
            with tc.tile_pool(name="pB", bufs=1) as pB, \
                 tc.tile_pool(name="ps", bufs=8, space="PSUM") as ps:
                x2 = [pB.tile([P, C], f32, tag=f"x2_{m}", name=f"x2_{m}")
                      for m in range(4)]
                w_ad_sb = pB.tile([P, 8, DOWN], f16, tag="wad", name="wad")
                w_au_sb = pB.tile([P, 2, C], f16, tag="wau", name="wau")

                # hidden activations: k-tiles 0:K8 in fp8 (DoubleRow with
                # w_mp8), the rest fp16. All fc weights are x32 (uniform
                # PSUM scale), descale 1/32 inside the GELU eviction.
                h1T8 = pB.tile([P, K8, TQ], f8, tag="h1T8", name="h1T8")
                h1T = pB.tile([P, 32 - K8, TQ], f16, tag="h1T", name="h1T")
                wfcs8, wfcs16 = [], []
                for ch in range(16):
                    wfc8 = wf8p.tile([P, 2, 2, 2 * P], f8, tag="wfc8", name="wfc8")
                    nc.gpsimd.dma_start(wfc8[:], w_fc8[ch])
                    wfcs8.append(wfc8)
                    wfc2 = wfcp.tile([P, 8 - J8, 2 * P], f16, tag="wfcm", name="wfcm")
                    nc.gpsimd.dma_start(wfc2[:], w_fc16[ch])
                    wfcs16.append(wfc2)

                def fc_tile(M, msl, tsl):
                    """one fc output tile M over token range tsl (ln2T m
                    slice msl); fp16 k-tiles first (only need ln2T), then
                    fp8 DoubleRow pairs (need the ln2T8 copy)."""
                    h2 = M % 2
                    pt = ps.tile([P, TQ], f32, tag="mm", name="mm")
                    n = (tsl.stop - tsl.start)
                    for k in range(J8, 8):
                        nc.tensor.matmul(
                            pt[:, 0:n], wfcs16[M // 2][:, k - J8, P * h2:P * (h2 + 1)],
                            ln2T[:, msl, k, :], start=(k == J8), stop=False)
                    for aj in range(J8 // 2):
                        nc.tensor.matmul(
                            pt[:, 0:n], wfcs8[M // 2][:, aj, :, P * h2:P * (h2 + 1)],
                            ln2T8[:, 2 * aj:2 * aj + 2, msl, :],
                            start=False, stop=(aj == J8 // 2 - 1), perf_mode=DR)
                    dst = (h1T8[:, M, tsl] if M < K8
                           else h1T[:, M - K8, tsl])
                    nc.scalar.activation(dst, pt[:, 0:n], AF.Gelu_apprx_tanh,
                                         bias=fcb_sb[:, M:M + 1],
                                         scale=1.0 / WS)

                # hybrid sweep: the first NSPLIT M tiles run in token halves
                # (the tg=0 half only needs ln2T m0/m1, bridging the
                # proj->LN2->transpose chain of m2/m3); the rest run at
                # N=512, the cheapest per-column shape on hardware.
                NSPLIT = 10
                for tg in range(2):
                    for M in range(NSPLIT):
                        fc_tile(M, slice(2 * tg, 2 * tg + 2),
                                slice(TQ // 2 * tg, TQ // 2 * (tg + 1)))
                for M in range(NSPLIT, 32):
                    fc_tile(M, slice(0, 4), slice(0, TQ))

                # mproj with 8 output tiles resident in PSUM; k 0:K8 as
                # fp8 DoubleRow pairs from h1T8, k K8:32 fp16 from h1T;
                # weights stream in chunks at PE consumption rate. All
                # weights x32 -> descale 1/32 in the eviction.
                def mproj_pass(ms):
                    pts = {(m, n2): ps.tile([P, TQ], f32, tag="mm", name="mm")
                           for m in ms for n2 in range(2)}
                    for ac in range(K8 // 4):
                        wmp8 = wmpp.tile([P, 2, 2, C], f8, tag="wmp8", name="wmp8")
                        nc.gpsimd.dma_start(wmp8[:], w_mp8[:, 2 * ac:2 * ac + 2, :, :])
                        for da in range(2):
                            a = 2 * ac + da
                            for m in ms:
                                for n2 in range(2):
                                    nc.tensor.matmul(
                                        pts[(m, n2)][:],
                                        h1T8[:, 2 * a:2 * a + 2, P * m:P * (m + 1)],
                                        wmp8[:, da, :, TQ * n2:TQ * (n2 + 1)],
                                        start=(a == 0), stop=False,
                                        perf_mode=DR, skip_group_check=True)
                    NK16 = (32 - K8) // 2
                    for kc in range(NK16):
                        wmp2 = wmpp.tile([P, 2, C], f16, tag="wmp2", name="wmp2")
                        nc.gpsimd.dma_start(wmp2[:], w_mp16[:, 2 * kc:2 * kc + 2, :])
                        # in the final chunk, finish m0/m1 first so their
                        # eviction -> transpose chains overlap the m2/m3
                        # matmuls instead of the adapter's start.
                        order = (lambda: [(m, dk) for m in ms for dk in range(2)]
                                 ) if kc == NK16 - 1 else (
                                 lambda: [(m, dk) for dk in range(2) for m in ms])
                        for m, dk in order():
                            k = 2 * kc + dk
                            for n2 in range(2):
                                nc.tensor.matmul(
                                    pts[(m, n2)][:],
                                    h1T[:, k, P * m:P * (m + 1)],
                                    wmp2[:, dk, TQ * n2:TQ * (n2 + 1)],
                                    start=False, stop=(k == 2 * NK16 - 1),
                                    skip_group_check=True)
                    # x2 is stored fp16 (ln2h) straight out of the STT --
                    # the adapter's final residual add re-materializes f32.
                    for m in ms:
                        for n2 in range(2):
                            nc.vector.scalar_tensor_tensor(
                                out=ln2h[m][:, TQ * n2:TQ * (n2 + 1)],
                                in0=pts[(m, n2)][:],
                                scalar=1.0 / WS,
                                in1=x1[m][:, TQ * n2:TQ * (n2 + 1)],
                                op0=ALU.mult, op1=ALU.add)
                        nc.sync.dma_start_transpose(ln2T[:, m, :, :], ln2h[m][:])

                mproj_pass((0, 1, 2, 3))

                nc.gpsimd.dma_start(w_ad_sb[:], w_ad[:])
                nc.gpsimd.dma_start(w_au_sb[:], w_au[:])

                # adapter, interleaved per token block: up(m) only needs
                # down-quarter tg=m, so each block drains to DRAM while the
                # next block computes. All fp16; tanh-GELU (same table set
                # as fc; exact-vs-tanh difference is ~5e-4 absolute, washed
                # out by the x0.02 ad_up weights).
                aT = pB.tile([P, 2, TQ], f16, tag="aT", name="aT")
                for m in range(4):
                    for M in range(2):
                        pt = ps.tile([P, TQ], f32, tag="mm", name="mm")
                        for k in range(8):
                            nc.tensor.matmul(
                                pt[:, 0:P], w_ad_sb[:, k, P * M:P * (M + 1)],
                                ln2T[:, m, k, :],
                                start=(k == 0), stop=(k == 7))
                        nc.scalar.activation(
                            aT[:, M, P * m:P * (m + 1)],
                            pt[:, 0:P], AF.Gelu_apprx_tanh,
                            bias=adb_sb[:, M:M + 1])
                    for n2 in range(2):
                        pt = ps.tile([P, TQ], f32, tag="mm", name="mm")
                        for k in range(2):
                            nc.tensor.matmul(
                                pt[:], aT[:, k, P * m:P * (m + 1)],
                                w_au_sb[:, k, TQ * n2:TQ * (n2 + 1)],
                                start=(k == 0), stop=(k == 1))
                        nc.vector.scalar_tensor_tensor(
                            out=x2[m][:, TQ * n2:TQ * (n2 + 1)], in0=pt[:],
                            scalar=1.0,
                            in1=ln2h[m][:, TQ * n2:TQ * (n2 + 1)],
                            op0=ALU.mult, op1=ALU.add)
                        eng = nc.sync if n2 == 0 else nc.scalar
                        eng.dma_start(
                            y_r[m][:, TQ * n2:TQ * (n2 + 1)],
                            x2[m][:, TQ * n2:TQ * (n2 + 1)])

    _split_excess_waits(nc)
    return nc


def _prep_inputs(inputs):
    f16 = ml_dtypes.float16 if hasattr(ml_dtypes, "float16") else np.float16
    f8 = ml_dtypes.float8_e4m3fn
    f32 = np.float32

    def q8(w):
        return np.clip(np.asarray(w, f32) * WS, -240.0, 240.0).astype(f8)

    x = np.ascontiguousarray(np.asarray(inputs["x"], f32))
    vf = np.asarray(inputs["visual_features"], f32)
    # collapsed cross-attention (uniform softmax over identical keys)
    ca_add = ((vf @ np.asarray(inputs["v_w"], f32)
               + np.asarray(inputs["v_b"], f32))
              @ np.asarray(inputs["ca_proj_w"], f32)
              + np.asarray(inputs["ca_proj_b"], f32))        # [B, C]

    # host-side LN1 (pure function of the input x; gain folded into attn_w)
    mu = x.mean(axis=-1, keepdims=True)
    var = np.square(x - mu).mean(axis=-1, keepdims=True)
    h_full = (x - mu) / np.sqrt(var + 1e-5)          # [B, T, C] f32

    # fold LN gains into the consuming weights
    g1 = np.asarray(inputs["ln1_g"], f32)[:, None]
    g2 = np.asarray(inputs["ln2_g"], f32)[:, None]
    attn_w = np.asarray(inputs["attn_w"], f32) * g1
    attn_b = np.asarray(inputs["attn_b"], f32)
    tri = np.triu(np.ones((P, P), f32))          # tri[k, q] = 1 iff k <= q

    def chunks2(w, nch):
        # [C_in, n_out] -> [nch, P, C_in//P, n_out//nch] contiguous per chunk
        cin, cout = w.shape
        return np.ascontiguousarray(
            w.reshape(cin // P, P, nch, cout // nch).transpose(2, 1, 0, 3))

    def kmaj(w):
        # [C_in, n_out] -> [P, C_in//P, n_out]
        cin, cout = w.shape
        return np.ascontiguousarray(
            w.reshape(cin // P, P, cout).transpose(1, 0, 2))

    def kmaj_pair(w):
        # [C_in, n_out] -> [P, C_in//256, 2, n_out]  (DoubleRow k-pairs)
        cin, cout = w.shape
        return np.ascontiguousarray(
            w.reshape(cin // 256, 2, P, cout).transpose(2, 0, 1, 3))

    wq = attn_w[:, :C]
    wk = attn_w[:, C:2 * C]
    wv_ = attn_w[:, 2 * C:]

    def dq(a):
        return np.asarray(a, f32)

    wv_pair = [kmaj_pair(wv_[:, n2 * TQ:(n2 + 1) * TQ]) for n2 in range(2)]
    wv8 = [q8(w) for w in wv_pair]
    # x1024 fp8 residual of the x32 fp8 v-weights (hi/lo correction)
    wv_lo = [np.clip((w - dq(w8) / WS) * (WS * WS), -240.0, 240.0)
             .astype(f8) for w, w8 in zip(wv_pair, wv8)]

    fc_w = np.asarray(inputs["fc_w"], f32) * g2
    # fc chunks: [16, P, 8, 256]; k-pairs 0:J8 fp8, rest fp16, all x32
    fc_ch = chunks2(fc_w, 16)
    w_fc8 = np.ascontiguousarray(
        fc_ch[:, :, 0:J8].reshape(16, P, J8 // 2, 2, 2 * P))
    mp_w = np.asarray(inputs["mproj_w"], f32)
    mp_k = kmaj(mp_w)            # [P, 32, C]
    w_mp8 = np.ascontiguousarray(mp_k[:, 0:K8].reshape(P, K8 // 2, 2, C))

    shared = {
        "w_qkv": np.stack([
            q8(kmaj_pair(np.concatenate(
                [wq[:, hp * P:(hp + 1) * P],
                 wk[:, hp * P:(hp + 1) * P]], axis=1)))
            for hp in range(8)], axis=0),
        "w_v": np.stack(wv8, axis=0),
        "w_v_lo": np.stack(wv_lo, axis=0),
        "w_pj": q8(kmaj_pair(np.asarray(inputs["attn_proj_w"], f32))),
        "w_pj16": kmaj(np.asarray(inputs["attn_proj_w"], f32)).astype(f16),
        "w_fc8": q8(w_fc8),
        "w_fc16": (fc_ch[:, :, J8:] * WS).astype(f16),
        "w_mp8": q8(w_mp8),
        "w_mp16": (mp_k[:, K8:] * WS).astype(f16),
        "w_ad": kmaj(np.asarray(inputs["ad_down_w"], f32)).astype(f16),
        "w_au": kmaj(np.asarray(inputs["ad_up_w"], f32)).astype(f16),
        "bqk_T": np.ascontiguousarray(
            (attn_b[:2 * C] * WS).reshape(16, P).T),
        "fcb_T": np.ascontiguousarray(
            np.asarray(inputs["fc_b"], f32).reshape(FF // P, P).T),
        "adb_T": np.ascontiguousarray(
            np.asarray(inputs["ad_down_b"], f32).reshape(DOWN // P, P).T),
        "tri": tri.astype(f16),
    }
    pj_bias = np.asarray(inputs["attn_proj_b"], f32)[None, :] + ca_add

    in_maps = []
    for c in range(NCORES):
        b, half = c // 2, c % 2
        xq = x[b, TQ * half:TQ * half + TQ]
        m = dict(shared)
        m["x_q"] = np.ascontiguousarray(xq + pj_bias[b][None, :])
        hkv = np.concatenate(
            [h_full[b, 0:TQ], h_full[b, TQ * half:TQ * half + TQ]], axis=0)
        # hT_in[p, a, e, i, f] = h[i*128+f, (2a+e)*128+p]
        hT8 = np.ascontiguousarray(
            np.clip(hkv, -240.0, 240.0)
            .reshape(8, P, 4, 2, P).transpose(4, 2, 3, 0, 1)).astype(f8)
        m["hT_in"] = hT8
        # x32 fp8 residual of tile i=4 (rows TQ*half : TQ*half+128)
        lo4 = (hkv.reshape(8, P, 4, 2, P).transpose(4, 2, 3, 0, 1)[:, :, :, 4, :]
               - dq(hT8[:, :, :, 4, :])) * WS
        m["h_lo4"] = np.clip(lo4, -240.0, 240.0).astype(f8)
        m["log_s"] = np.array([[0.0 if half == 1 else NEG]], f32)
        in_maps.append(m)
    return in_maps


def _bust_stale_neff_cache(nc):
    """The PJRT/neuronxcc compile cache keys on the HLO wrapper module,
    which does NOT change when only this bass program's instructions
    change (same I/O signature) -- a stale NEFF would silently run.
    Wipe the cache whenever the built program's hash differs from the
    marker left by the previous build."""
    import hashlib
    import os
    import shutil
    h = hashlib.md5()
    for fn in nc.m.functions:
        for bb in fn.blocks:
            for ins in bb.instructions:
                h.update(str(ins).encode())
    sig = h.hexdigest()
    cdir = os.path.expanduser("~/.neuron-compile-cache")
    marker = os.path.join(cdir, "bass_kernel_sig.txt")
    try:
        with open(marker) as f:
            if f.read().strip() == sig:
                return
    except OSError:
        pass
    shutil.rmtree(cdir, ignore_errors=True)
    os.makedirs(cdir, exist_ok=True)
    with open(marker, "w") as f:
        f.write(sig)


def kernel(**inputs) -> np.ndarray:
    from concourse.bass_utils import run_bass_kernel_spmd

    if "nc" not in _CACHE:
        _CACHE["nc"] = _build_program()
        _bust_stale_neff_cache(_CACHE["nc"])
    nc = _CACHE["nc"]

    in_maps = _prep_inputs(inputs)
    res = run_bass_kernel_spmd(nc, in_maps, list(range(NCORES)))

    out = np.zeros((B, T, C), np.float32)
    for c in range(NCORES):
        b, half = c // 2, c % 2
        out[b, TQ * half:TQ * half + TQ] = res.results[c]["y"]
    return out


# revision 55
# speedup vs baseline: 3.0035x; 3.0035x over previous
"""Trainium2 Bass kernel for nn_Block_47502338294589 (dense transformer block).

Block (B=4, T=1024, C=1024, H=16 heads, D=64):
    x += causal_selfattn(LN1(x)) @ attn_proj
    x += crossattn(x, visual_features) @ ca_proj
    x += MLP(LN2(x))          (tanh GELU, 4C hidden)
    x += adapter(x)           (exact GELU, 256 hidden)

Host-side algebra (pure functions of the inputs):
  - cross-attention keys/values are identical at every position, so its
    softmax is uniform and the whole branch collapses to a per-batch
    additive vector, folded into the attn-proj residual bias;
  - LN gains fold into the consuming weights; LN1 itself is computed on
    the host and shipped pre-transposed (fp8) so the device's first
    matmul starts as soon as the first DMA chunk lands;
  - every weight matrix is pre-rearranged so each device DMA reads
    fully contiguous memory (sub-512B runs pay a 2x DMA penalty).

Precision: residual stream / LN stats / softmax sums fp32; S and O
matmuls + MLP in fp16 (same PE rate as bf16, 8x less rounding error);
q/k/v projections, attn-proj and ad_up in fp8e4 with
perf_mode=DoubleRow (2 contraction tiles per matmul -> ~2x PE rate).
fp8 weights are pre-scaled x32 on the host (dodges e4m3 subnormals at
|w|~0.02); the descale is free: folded into the softmax exp scale
(1/32^2) and the PSUM-eviction scalar multipliers.

Sharding: sequence-parallel, 8 cores = 4 batches x 2 sequence halves,
no collectives. Core c computes the 512 query rows [512*(c%2), ...) of
batch c//2, with keys [A | B]: A = rows 0:512 masked per-core via a
log-bias input, B = own rows with compile-time causal structure.

Device schedule:
  - attention is ACT(exp)-throughput-bound relative to its own matmuls,
    so q/k projections for head-pair hp+1 are emitted interleaved
    between the S and O matmuls of head-pair hp (software pipeline);
    V is computed for all heads upfront in N=512 DoubleRow matmuls;
  - all weights stream on the gpsimd SWDGE queue (separate pipe from
    the HWDGE used by activations/transposes);
  - LN2 row-sums ride the proj evictions' accum_out; sum(x^2) is one
    ACT Square pass; rsqrt runs as exp(-0.5*ln(var+eps)) and the
    normalize as ACT Identity(x*rstd - mean*rstd), so ACT stays in the
    natural_log_exp table set and the DVE only does [P,1] algebra; the
    adapter uses tanh-GELU so only one table switch happens (exp->gelu);
  - fc/mproj run half fp8-DoubleRow, half fp16 (k-split): full-fp8 MLP
    would blow the 2e-2 error gate; the early causal rows additionally
    get hi/lo-corrected V and an fp16 m=0 attn-proj (their attention
    output barely averages, so fp8 noise there dominated the absmax);
  - the feature-major transposes needed by fc/adapter run on the SP
    HWDGE queue, off the ACT datapath.

Self-contained: hardcodes shapes; needs numpy/ml_dtypes + concourse.
"""

import numpy as np
import ml_dtypes

B, T, C, H, D = 4, 1024, 1024, 16, 64
TQ = 512            # query rows per core
TKV = 1024          # A (512) + B (512) key rows per core
FF = 4 * C
DOWN = 256
P = 128
NCORES = 8
NEG = -30000.0      # exp(x + NEG) == 0 in fp32
WS = 32.0           # fp8 weight pre-scale
EXPS = 0.125 / (WS * WS)   # softmax scale incl. q/k fp8 descale
J8 = 4              # fc k-tiles (of 8) in fp8   (fraction of C)
K8 = 16             # mproj k-tiles (of 32) in fp8 (fraction of 4C)

_CACHE = {}


# --------------------------------------------------------------------------
# walrus workaround: setupSyncWait accepts at most 2 sync-wait commands per
# instruction (and lowering may add one of its own), while Tile's semaphore
# pass can attach more. Hoist excess waits onto same-engine NoOps placed
# immediately before the offending instruction; in-order execution keeps
# the semantics identical.
def _split_excess_waits(nc, max_waits=1):
    import concourse.mybir as mybir
    n_new = 0
    for fn in nc.m.functions:
        for bb in fn.blocks:
            out, changed = [], False
            for ins in bb.instructions:
                si = ins.sync_info
                if si is not None and si.on_wait is not None \
                        and len(si.on_wait) > max_waits:
                    waits = list(si.on_wait)
                    extra, keep = waits[:-max_waits], waits[-max_waits:]
                    for j in range(0, len(extra), max_waits):
                        n_new += 1
                        out.append(mybir.InstNoOp(
                            name=f"I-waitsplit-{n_new}",
                            engine=ins.engine,
                            bass_nofuse=True,
                            sync_info=mybir.SyncInfo(
                                on_wait=extra[j:j + max_waits], on_update=[]),
                        ))
                    si.on_wait = keep
                    ins.sync_info = si
                    changed = True
                out.append(ins)
            if changed:
                bb.instructions = out
    return n_new


def _build_program():
    import concourse.bass as bass
    import concourse.mybir as mybir
    from concourse.tile import TileContext

    dt = mybir.dt
    f32, f16, f8 = dt.float32, dt.float16, dt.float8e4
    AF = mybir.ActivationFunctionType
    ALU = mybir.AluOpType
    DR = mybir.MatmulPerfMode.DoubleRow

    nc = bass.Bass()

    def din(name, shape, dtype=f32):
        return nc.dram_tensor(name, shape, dtype, kind="ExternalInput")

    x_q = din("x_q", [TQ, C])            # query rows, ca/proj bias pre-added
    # hT_in[p, a, e, i, f] = LN1(x_kv)[i*128+f, (2a+e)*128+p] in fp8 --
    # feature-pair-major so DoubleRow matmuls slice [p, 2, N] 3D APs.
    hT_in = din("hT_in", [P, 4, 2, 8, P], f8)
    # x32 fp8 residual of hT tile i=4 (this core's first 128 own rows):
    # feeds the hi/lo-corrected V projection for the early causal rows,
    # whose attention output barely averages and is fp8-sensitive.
    h_lo4 = din("h_lo4", [P, 4, 2, P], f8)
    log_s = din("log_s", [1, 1])         # 0.0 (A visible) or NEG (A masked)
    # weights, host-prearranged for contiguous DMA (see _prep_inputs)
    # per-head-pair fused q/k weights: cols = [q 128 | k 128], x32 fp8
    w_qkv = din("w_qkv", [8, P, 4, 2, 2 * P], f8)  # [hp, p, a, e, 256]
    w_v = din("w_v", [2, P, 4, 2, TQ], f8)         # [n2, p, a, e, c] x32
    w_v_lo = din("w_v_lo", [2, P, 4, 2, TQ], f8)   # x1024 fp8 residual
    w_pj = din("w_pj", [P, 4, 2, C], f8)           # [p, a, e, c] x32
    w_pj16 = din("w_pj16", [P, 8, C], f16)         # fp16 copy (m=0 proj)
    w_fc8 = din("w_fc8", [16, P, 2, 2, 2 * P], f8)   # k 0:J8 x32 fp8 pairs
    w_fc16 = din("w_fc16", [16, P, 8 - J8, 2 * P], f16)  # k J8:8 x32
    w_mp8 = din("w_mp8", [P, K8 // 2, 2, C], f8)   # k 0:K8 x32 fp8 pairs
    w_mp16 = din("w_mp16", [P, 32 - K8, C], f16)   # k K8:32 x32
    w_ad = din("w_ad", [P, 8, DOWN], f16)          # [p, k, c]
    w_au = din("w_au", [P, 2, C], f16)             # [p, k, c]
    bqk_T = din("bqk_T", [P, 16])        # attn_b[:2C] partition-major (x32)
    fcb_T = din("fcb_T", [P, FF // P])   # fc_b partition-major
    adb_T = din("adb_T", [P, DOWN // P])  # ad_down_b partition-major
    tri = din("tri", [P, P], f16)        # tri[k, q] = 1 if k <= q
    y_out = nc.dram_tensor("y", [TQ, C], f32, kind="ExternalOutput")

    x_q_r = x_q.rearrange("(i p) c -> i p c", p=P)
    y_r = y_out.rearrange("(i p) c -> i p c", p=P)

    with TileContext(nc) as tc:
        with tc.tile_pool(name="res", bufs=1) as res, \
             tc.tile_pool(name="scr", bufs=3) as scr, \
             tc.tile_pool(name="wfcp", bufs=12) as wfcp, \
             tc.tile_pool(name="wf8p", bufs=16) as wf8p, \
             tc.tile_pool(name="wmpp", bufs=4) as wmpp:

            # hT arrives pre-normalized/pre-transposed fp8 from the host;
            # DMAs head the SP queue, B tiles (i=4:8) per k-pair first so
            # the prologue V/q/k matmuls start as chunks land.
            hT = res.tile([P, 4, 2, 8, P], f8, tag="hT", name="hT")
            for a in range(4):
                nc.sync.dma_start(hT[:, a, :, 4:8, :], hT_in[:, a, :, 4:8, :])
            nc.sync.dma_start(hT[:, 0:2, :, 0:4, :], hT_in[:, 0:2, :, 0:4, :])
            nc.sync.dma_start(hT[:, 2:4, :, 0:4, :], hT_in[:, 2:4, :, 0:4, :])

            # ---- constants -------------------------------------------------
            logs_b = res.tile([P, 1], f32, tag="logs", name="logs")
            nc.sync.dma_start(logs_b[:], log_s[:].to_broadcast((P, 1)))
            bqk_sb = res.tile([P, 16], f32, tag="bqk", name="bqk")
            nc.sync.dma_start(bqk_sb[:], bqk_T[:])
            fcb_sb = res.tile([P, FF // P], f32, tag="fcb", name="fcb")
            nc.sync.dma_start(fcb_sb[:], fcb_T[:])
            adb_sb = res.tile([P, DOWN // P], f32, tag="adb", name="adb")
            nc.sync.dma_start(adb_sb[:], adb_T[:])
            tri_sb = res.tile([P, P], f16, tag="tri", name="tri")
            nc.sync.dma_start(tri_sb[:], tri[:])
            ones_sb = res.tile([1, 64], f16, tag="ones", name="ones")
            nc.vector.memset(ones_sb[:], 1.0)
            eps_sb = res.tile([P, 1], f32, tag="eps", name="eps")
            nc.vector.memset(eps_sb[:], 1e-5)

            x1 = [res.tile([P, C], f32, tag=f"x1_{m}", name=f"x1_{m}") for m in range(4)]
            ln2h = [res.tile([P, C], f16, tag=f"l2h{m}", name=f"l2h{m}")
                    for m in range(4)]
            # ln2T_all[p, m, j, f] = ln2h[m][f, j*128+p]  (batched transpose out)
            ln2T = res.tile([P, 4, 8, P], f16, tag="l2T", name="l2T")
            # fp8 copy of ln2T's first J8 feature tiles, j-major so a
            # DoubleRow pair slice is a 3D [p, 2, 512] AP (j stride 512).
            ln2T8 = res.tile([P, J8, 4, P], f8, tag="l2T8", name="l2T8")

            def layernorm_finish(x_ap, out_16, sx, sq):
                """out = (x - mean)*rsqrt(var+eps) given sx[P,2] = per-half
                row sums of x (from the eviction STTs' accum_out) and
                sq[P,1] = row sum of x^2 (one full-width ACT Square).
                rsqrt = exp(-0.5*ln(var+eps)) keeps ACT in the exp/ln
                table set; the [P,C] normalize runs on ACT as
                Identity(x*rstd + (-mean*rstd)) -- the DVE only does tiny
                [P,1] algebra."""
                t = scr.tile([P, 2], f32, tag="ln_t", name="ln_t")
                nc.vector.tensor_tensor(t[:, 0:1], sx[:, 0:1], sx[:, 1:2],
                                        ALU.add)
                nc.vector.tensor_tensor(t[:, 1:2], t[:, 0:1], t[:, 0:1],
                                        ALU.mult)
                u = scr.tile([P, 1], f32, tag="ln_u", name="ln_u")
                # u = sum(x^2) - (sum x)^2/C ; var = u/C
                nc.vector.scalar_tensor_tensor(
                    out=u[:], in0=t[:, 1:2], scalar=-1.0 / C, in1=sq[:],
                    op0=ALU.mult, op1=ALU.add)
                lnv = scr.tile([P, 1], f32, tag="ln_lnv", name="ln_lnv")
                nc.scalar.activation(lnv[:], u[:], AF.Ln,
                                     bias=eps_sb[:], scale=1.0 / C)
                rstd = scr.tile([P, 1], f32, tag="ln_rstd", name="ln_rstd")
                nc.scalar.activation(rstd[:], lnv[:], AF.Exp, scale=-0.5)
                nb = scr.tile([P, 1], f32, tag="ln_nb", name="ln_nb")
                nc.vector.scalar_tensor_tensor(
                    out=nb[:], in0=t[:, 0:1], scalar=-1.0 / C, in1=rstd[:],
                    op0=ALU.mult, op1=ALU.mult)
                nc.scalar.activation(out_16, x_ap, AF.Identity,
                                     bias=nb[:], scale=rstd[:])

            # =========== phase A: LN1, fused qkv+attention, attn-proj ======
            # Attention is ACT(exp)-bound relative to its own matmuls, so
            # qkv production is fused into the per-head-pair loop: PE
            # computes hp's q/k/v and S/O while ACT streams previous exps.
            with tc.tile_pool(name="pA", bufs=1) as pA, \
                 tc.tile_pool(name="wqkp", bufs=3) as wqkp, \
                 tc.tile_pool(name="wvp", bufs=2) as wvp, \
                 tc.tile_pool(name="hpp", bufs=2) as hpp, \
                 tc.tile_pool(name="psQ", bufs=1, space="PSUM") as psQ, \
                 tc.tile_pool(name="psK", bufs=2, space="PSUM") as psK, \
                 tc.tile_pool(name="psS", bufs=3, space="PSUM") as psS, \
                 tc.tile_pool(name="psO", bufs=2, space="PSUM") as psO:
                # V shares psS's three banks (its phase precedes the S/O
                # loop); psS bufs=3 lets S(kt+1) run ahead of exp(kt) so
                # the ACT exp stream -- the attention-loop bottleneck --
                # never starves.
                psV = psS
                oT = pA.tile([P, 4, 2, TQ], f8, tag="oT", name="oT")
                # fp16 copy of oT's first 128 token columns: proj for the
                # early causal rows runs in fp16 (fp8 o there is the
                # dominant error source -- little softmax averaging).
                oT16 = pA.tile([P, 8, P], f16, tag="oT16", name="oT16")
                v_sb = pA.tile([P, 8, H, 65], f16, tag="vsb", name="vsb")
                w_pj_sb = pA.tile([P, 4, 2, C], f8, tag="wpj", name="wpj")
                w_pj16_sb = pA.tile([P, 8, C], f16, tag="wpj16", name="wpj16")
                h_lo_sb = pA.tile([P, 4, 2, P], f8, tag="hlo", name="hlo")
                nc.sync.dma_start(h_lo_sb[:], h_lo4[:])

                # --- software-pipelined per-head-pair qkv + attention ---
                # DoubleRow q/k/v: contraction pair a covers feature tiles
                # (2a, 2a+1); four a-steps replace eight k-steps.
                def alloc_hp(hp):
                    t = {}
                    t["wch"] = wqkp.tile([P, 4, 2, 2 * P], f8, tag="wqkv", name="wqkv")
                    nc.gpsimd.dma_start(t["wch"][:, 0:2, :, :], w_qkv[hp, :, 0:2, :, :])
                    nc.gpsimd.dma_start(t["wch"][:, 2:4, :, :], w_qkv[hp, :, 2:4, :, :])
                    t["qT"] = hpp.tile([P, TQ], f16, tag="qT", name="qT")
                    t["kT"] = hpp.tile([P, TKV], f16, tag="kT", name="kT")
                    t["pq"] = psQ.tile([P, TQ], f32, tag="q", name="q")
                    t["pk0"] = psK.tile([P, TQ], f32, tag="k", name="k")
                    t["pk1"] = psK.tile([P, TQ], f32, tag="k", name="k")
                    return t

                def emit_qk(hp, t, a, which):
                    st, sp = (a == 0), (a == 3)
                    wch = t["wch"]
                    if which == "q":
                        nc.tensor.matmul(t["pq"][:], wch[:, a, :, 0:P],
                                         hT[:, a, :, 4:8, :], start=st, stop=sp,
                                         perf_mode=DR)
                        if sp:
                            nc.vector.tensor_scalar_add(
                                t["qT"][:], t["pq"][:], bqk_sb[:, hp:hp + 1])
                    elif which == "k0":
                        nc.tensor.matmul(t["pk0"][:], wch[:, a, :, P:2 * P],
                                         hT[:, a, :, 0:4, :], start=st, stop=sp,
                                         perf_mode=DR)
                        if sp:
                            nc.vector.tensor_scalar_add(
                                t["kT"][:, 0:TQ], t["pk0"][:],
                                bqk_sb[:, 8 + hp:9 + hp])
                    else:
                        nc.tensor.matmul(t["pk1"][:], wch[:, a, :, P:2 * P],
                                         hT[:, a, :, 4:8, :], start=st, stop=sp,
                                         perf_mode=DR)
                        if sp:
                            nc.vector.tensor_scalar_add(
                                t["kT"][:, TQ:TKV], t["pk1"][:],
                                bqk_sb[:, 8 + hp:9 + hp])

                def emit_qkv_step(hp, t, a):
                    emit_qk(hp, t, a, "q")
                    emit_qk(hp, t, a, "k0")
                    emit_qk(hp, t, a, "k1")

                # prologue: hp0's q starts as soon as the B-chunks of hT
                # land (before V -- it only needs the same chunks), then
                # batched V for ALL head pairs in N=512 DoubleRow matmuls.
                # m=4 (this core's first 128 own rows) gets an extra hi/lo
                # correction group after the main sweep: h8@w_lo (x1024) +
                # h_lo@w8 (x1024), folded into v_sb with a x1/32 add --
                # fp16-grade v for the keys the early causal rows attend
                # to. V psums share the psS pool (tag "S").
                nc.vector.memset(v_sb[:, :, :, 64:65], 1.0)
                cur0 = alloc_hp(0)
                for a in range(4):
                    emit_qk(0, cur0, a, "q")
                wvns = []
                for n2 in range(2):
                    wvn = wvp.tile([P, 4, 2, TQ], f8, tag="wvn", name="wvn")
                    nc.gpsimd.dma_start(wvn[:, 0:2, :, :], w_v[n2, :, 0:2, :, :])
                    nc.gpsimd.dma_start(wvn[:, 2:4, :, :], w_v[n2, :, 2:4, :, :])
                    wvns.append(wvn)
                for n2 in range(2):
                    wvn = wvns[n2]
                    for m in (4, 5, 6, 7, 0, 1, 2, 3):
                        pv = psV.tile([P, TQ], f32, tag="S", name="pv")
                        for a in range(4):
                            nc.tensor.matmul(
                                pv[:], hT[:, a, :, m, :], wvn[:, a, :, :],
                                start=(a == 0), stop=(a == 3), perf_mode=DR)
                        # ACT is idle during the V phase; evicting there
                        # frees the shared psS slots at PE rate instead of
                        # queueing behind DVE.
                        nc.scalar.activation(
                            v_sb[:, m, 8 * n2:8 * (n2 + 1), 0:64],
                            pv[:].rearrange("p (h d) -> p h d", d=64),
                            AF.Copy)

                # prologue k for hp=0
                cur = cur0
                for a in range(4):
                    emit_qk(0, cur, a, "k1")
                for a in range(4):
                    emit_qk(0, cur, a, "k0")

                # m=4 hi/lo correction (v_sb[4] is first consumed at kt=4
                # of hp0, ~10us after this)
                wvlos = []
                for n2 in range(2):
                    wvlo = wvp.tile([P, 4, 2, TQ], f8, tag="wvlo", name="wvlo")
                    nc.gpsimd.dma_start(wvlo[:], w_v_lo[n2])
                    wvlos.append(wvlo)
                for n2 in range(2):
                    pvc = psV.tile([P, TQ], f32, tag="S", name="pvc")
                    for a in range(4):
                        nc.tensor.matmul(
                            pvc[:], hT[:, a, :, 4, :], wvlos[n2][:, a, :, :],
                            start=(a == 0), stop=False, perf_mode=DR)
                    for a in range(4):
                        nc.tensor.matmul(
                            pvc[:], h_lo_sb[:, a, :, :], wvns[n2][:, a, :, :],
                            start=False, stop=(a == 3), perf_mode=DR)
                    nc.vector.scalar_tensor_tensor(
                        out=v_sb[:, 4, 8 * n2:8 * (n2 + 1), 0:64],
                        in0=pvc[:].rearrange("p (h d) -> p h d", d=64),
                        scalar=1.0 / WS,
                        in1=v_sb[:, 4, 8 * n2:8 * (n2 + 1), 0:64],
                        op0=ALU.mult, op1=ALU.add)
                with tc.tile_wait_until(0.03):
                    nc.gpsimd.dma_start(w_pj_sb[:], w_pj[:])
                    nc.gpsimd.dma_start(w_pj16_sb[:], w_pj16[:])
                    for m in range(4):
                        nc.gpsimd.dma_start(x1[m][:], x_q_r[m])

                def emit_proj_m(m, a_lo, a_hi, sx=None):
                    """proj partial over hp-pair range [a_lo, a_hi) for
                    token tile m, evict-added into x1. accum_out (row
                    sums for LN2) only on the final partial, whose STT
                    output is the completed x1."""
                    for n2 in range(2):
                        pool, ptag = (psS, "S") if n2 == 0 else (psO, "O")
                        pt = pool.tile([P, TQ], f32, tag=ptag, name="Spj")
                        if m == 0:
                            for k in range(2 * a_lo, 2 * a_hi):
                                nc.tensor.matmul(
                                    pt[:, 0:TQ], oT16[:, k, :],
                                    w_pj16_sb[:, k, TQ * n2:TQ * (n2 + 1)],
                                    start=(k == 2 * a_lo), stop=(k == 2 * a_hi - 1))
                        else:
                            for a in range(a_lo, a_hi):
                                nc.tensor.matmul(
                                    pt[:], oT[:, a, :, P * m:P * (m + 1)],
                                    w_pj_sb[:, a, :, TQ * n2:TQ * (n2 + 1)],
                                    start=(a == a_lo), stop=(a == a_hi - 1),
                                    perf_mode=DR)
                        nc.vector.scalar_tensor_tensor(
                            out=x1[m][:, TQ * n2:TQ * (n2 + 1)], in0=pt[:],
                            scalar=(1.0 / WS if m == 0 else 1.0 / (WS * WS)),
                            in1=x1[m][:, TQ * n2:TQ * (n2 + 1)],
                            op0=ALU.mult, op1=ALU.add,
                            accum_out=(sx[:, n2:n2 + 1] if sx is not None
                                       else None))
                    return sx

                for hp in range(8):
                    nxt = alloc_hp(hp + 1) if hp + 1 < 8 else None
                    qT, kT = cur["qT"], cur["kT"]
                    pO = [psO.tile([65, TQ], f32, tag="O", name="O")
                          for _ in range(2)]
                    for kt in range(8):
                        is_b = kt >= 4
                        q0 = P * (kt - 4) if is_b else 0
                        nq = TQ - q0
                        ksl = slice(P * kt, P * (kt + 1))
                        pS = [None, None]
                        for hh in range(2):
                            rows = slice(64 * hh, 64 * (hh + 1))
                            pS[hh] = psS.tile([P, TQ], f32, tag="S", name="S")
                            nc.tensor.matmul(
                                pS[hh][:, 0:nq], kT[rows, ksl],
                                qT[rows, q0:TQ], start=True, stop=True)
                        # next hp's q/k in the FIRST four kts: their DVE
                        # evictions then precede this hp's oT-normalize
                        # chain in the in-order DVE queue, so the next
                        # hp's S (and the ACT exp stream) start without
                        # the ~3us eviction-behind-normalize stall.
                        if nxt is not None and kt < 4:
                            emit_qkv_step(hp + 1, nxt, kt)
                        pT = scr.tile([P, 2, TQ], f16, tag="pT", name="pT")
                        for hh in range(2):
                            if is_b:
                                nc.scalar.activation(
                                    pT[:, hh, 0:nq], pS[hh][:, 0:nq],
                                    AF.Exp, scale=EXPS)
                                nc.vector.tensor_mul(
                                    pT[:, hh, 0:P], pT[:, hh, 0:P], tri_sb[:])
                            else:
                                nc.scalar.activation(
                                    pT[:, hh, 0:nq], pS[hh][:, 0:nq],
                                    AF.Exp, scale=EXPS, bias=logs_b[:])
                        for hh in range(2):
                            nc.tensor.matmul(
                                pO[hh][:, q0:TQ],
                                v_sb[:, kt, 2 * hp + hh, :], pT[:, hh, 0:nq],
                                start=(kt == 0), stop=(kt == 7),
                                skip_group_check=True)
                    for hh in range(2):
                        sums = scr.tile([1, TQ], f16, tag="sums", name="sums",
                                        bufs=2)
                        nc.vector.tensor_copy(sums[:], pO[hh][64:65, :])
                        # pR lives in the psK rotation: in psS it would
                        # hold an S slot hostage until the DVE reciprocal
                        # drains it (stalling the next hp's S->exp
                        # stream); in psO it deadlocks against its own pO
                        # source. psK's tiles are evicted well before the
                        # hp ends, and the displaced wait lands on the
                        # hp+2 q/k stream, which has a whole hp of slack.
                        pR = psK.tile([P, TQ], f32, tag="k", name="pR")
                        nc.tensor.matmul(pR[0:64, :], ones_sb[:], sums[:],
                                         start=True, stop=True)
                        rbc = scr.tile([64, TQ], f16, tag="rbc", name="rbc", bufs=2)
                        with nc.allow_low_precision(reason="1/sum in fp16; "
                                                    "sums are O(1e3), fine"):
                            nc.vector.reciprocal(rbc[:], pR[0:64, :])
                        # oT16 first: proj m0 (the bridge head) waits on
                        # it, while the fp8 oT feeds the later DR tiles.
                        nc.vector.tensor_mul(
                            oT16[64 * hh:64 * (hh + 1), hp, :],
                            pO[hh][0:64, 0:P], rbc[:, 0:P])
                        nc.vector.tensor_mul(
                            oT[64 * hh:64 * (hh + 1), hp // 2, hp % 2, :],
                            pO[hh][0:64, :], rbc[:])
                    cur = nxt

                # attn projection + residual into x1 (x_q has the collapsed
                # cross-attention + proj biases pre-added on the host).
                # m=0 (the early causal rows) runs fp16 from oT16; m>=1
                # runs fp8 DoubleRow with descale 1/1024 in the eviction.
                # LN2(m) is inline; proj n2=1 uses the psO pool so two
                # m-iterations of evictions can be in flight while the
                # DVE works through the LN chain.
                def emit_ln_m(m, sx):
                    sq = scr.tile([P, 1], f32, tag="ln_sq", name="ln_sq")
                    # discarded payload; only accum_out matters
                    sqd = scr.tile([P, C], f16, tag="ln_sqd", name="ln_sqd",
                                   bufs=2)
                    nc.scalar.activation(sqd[:], x1[m][:], AF.Square,
                                         accum_out=sq[:])
                    layernorm_finish(x1[m][:], ln2h[m][:], sx, sq)
                    nc.sync.dma_start_transpose(ln2T[:, m, :, :], ln2h[m][:])
                    nc.vector.tensor_copy(ln2T8[:, :, m, :],
                                          ln2T[:, m, 0:J8, :])

                # finals only (hp-pair a=3): a=0..3's partials were
                # emitted inside the hp loop where PE/DVE had slack.
                # Emission order = engine-queue order: m0/m1's LN chains
                # go ahead of m2/m3's bulk work so fc's first token-half
                # sweep (which waits on exactly the m0/m1 transposes)
                # isn't head-of-line-blocked behind m2/m3 Square passes.
                sx0 = scr.tile([P, 2], f32, tag="ln_sx", name="ln_sx", bufs=4)
                emit_proj_m(0, 0, 4, sx0)
                emit_ln_m(0, sx0)
                sx1 = scr.tile([P, 2], f32, tag="ln_sx", name="ln_sx", bufs=4)
                emit_proj_m(1, 0, 4, sx1)
                emit_ln_m(1, sx1)
                sx2 = scr.tile([P, 2], f32, tag="ln_sx", name="ln_sx", bufs=4)
                sx3 = scr.tile([P, 2], f32, tag="ln_sx", name="ln_sx", bufs=4)
                emit_proj_m(2, 0, 4, sx2)
                emit_proj_m(3, 0, 4, sx3)
                emit_ln_m(2, sx2)
                emit_ln_m(3, sx3)

            # =========== phase B: LN2, MLP, adapter ========================
            with tc.tile_pool(name="pB", bufs=1) as pB, \
                 tc.tile_pool(name="ps", bufs=8, space="PSUM") as ps:
                x2 = [pB.tile([P, C], f32, tag=f"x2_{m}", name=f"x2_{m}")
                      for m in range(4)]
                w_ad_sb = pB.tile([P, 8, DOWN], f16, tag="wad", name="wad")
                w_au_sb = pB.tile([P, 2, C], f16, tag="wau", name="wau")

                # hidden activations: k-tiles 0:K8 in fp8 (DoubleRow with
                # w_mp8), the rest fp16. All fc weights are x32 (uniform
                # PSUM scale), descale 1/32 inside the GELU eviction.
                h1T8 = pB.tile([P, K8, TQ], f8, tag="h1T8", name="h1T8")
                h1T = pB.tile([P, 32 - K8, TQ], f16, tag="h1T", name="h1T")
                wfcs8, wfcs16 = [], []
                for ch in range(16):
                    wfc8 = wf8p.tile([P, 2, 2, 2 * P], f8, tag="wfc8", name="wfc8")
                    nc.gpsimd.dma_start(wfc8[:], w_fc8[ch])
                    wfcs8.append(wfc8)
                    wfc2 = wfcp.tile([P, 8 - J8, 2 * P], f16, tag="wfcm", name="wfcm")
                    nc.gpsimd.dma_start(wfc2[:], w_fc16[ch])
                    wfcs16.append(wfc2)

                def fc_tile(M, msl, tsl):
                    """one fc output tile M over token range tsl (ln2T m
                    slice msl); fp16 k-tiles first (only need ln2T), then
                    fp8 DoubleRow pairs (need the ln2T8 copy)."""
                    h2 = M % 2
                    pt = ps.tile([P, TQ], f32, tag="mm", name="mm")
                    n = (tsl.stop - tsl.start)
                    for k in range(J8, 8):
                        nc.tensor.matmul(
                            pt[:, 0:n], wfcs16[M // 2][:, k - J8, P * h2:P * (h2 + 1)],
                            ln2T[:, msl, k, :], start=(k == J8), stop=False)
                    for aj in range(J8 // 2):
                        nc.tensor.matmul(
                            pt[:, 0:n], wfcs8[M // 2][:, aj, :, P * h2:P * (h2 + 1)],
                            ln2T8[:, 2 * aj:2 * aj + 2, msl, :],
                            start=False, stop=(aj == J8 // 2 - 1), perf_mode=DR)
                    dst = (h1T8[:, M, tsl] if M < K8
                           else h1T[:, M - K8, tsl])
                    nc.scalar.activation(dst, pt[:, 0:n], AF.Gelu_apprx_tanh,
                                         bias=fcb_sb[:, M:M + 1],
                                         scale=1.0 / WS)

                # hybrid sweep: the first NSPLIT M tiles run in token halves
                # (the tg=0 half only needs ln2T m0/m1, bridging the
                # proj->LN2->transpose chain of m2/m3); the rest run at
                # N=512, the cheapest per-column shape on hardware.
                NSPLIT = 6
                for tg in range(2):
                    for M in range(NSPLIT):
                        fc_tile(M, slice(2 * tg, 2 * tg + 2),
                                slice(TQ // 2 * tg, TQ // 2 * (tg + 1)))
                for M in range(NSPLIT, 32):
                    fc_tile(M, slice(0, 4), slice(0, TQ))

                # mproj with 8 output tiles resident in PSUM; k 0:K8 as
                # fp8 DoubleRow pairs from h1T8, k K8:32 fp16 from h1T;
                # weights stream in chunks at PE consumption rate. All
                # weights x32 -> descale 1/32 in the eviction.
                def mproj_pass(ms):
                    pts = {(m, n2): ps.tile([P, TQ], f32, tag="mm", name="mm")
                           for m in ms for n2 in range(2)}
                    for ac in range(K8 // 4):
                        wmp8 = wmpp.tile([P, 2, 2, C], f8, tag="wmp8", name="wmp8")
                        nc.gpsimd.dma_start(wmp8[:], w_mp8[:, 2 * ac:2 * ac + 2, :, :])
                        for da in range(2):
                            a = 2 * ac + da
                            for m in ms:
                                for n2 in range(2):
                                    nc.tensor.matmul(
                                        pts[(m, n2)][:],
                                        h1T8[:, 2 * a:2 * a + 2, P * m:P * (m + 1)],
                                        wmp8[:, da, :, TQ * n2:TQ * (n2 + 1)],
                                        start=(a == 0), stop=False,
                                        perf_mode=DR, skip_group_check=True)
                    NK16 = (32 - K8) // 2
                    for kc in range(NK16):
                        wmp2 = wmpp.tile([P, 2, C], f16, tag="wmp2", name="wmp2")
                        nc.gpsimd.dma_start(wmp2[:], w_mp16[:, 2 * kc:2 * kc + 2, :])
                        # in the final chunk, finish m0/m1 first so their
                        # eviction -> transpose chains overlap the m2/m3
                        # matmuls instead of the adapter's start.
                        order = (lambda: [(m, dk) for m in ms for dk in range(2)]
                                 ) if kc == NK16 - 1 else (
                                 lambda: [(m, dk) for dk in range(2) for m in ms])
                        for m, dk in order():
                            k = 2 * kc + dk
                            for n2 in range(2):
                                nc.tensor.matmul(
                                    pts[(m, n2)][:],
                                    h1T[:, k, P * m:P * (m + 1)],
                                    wmp2[:, dk, TQ * n2:TQ * (n2 + 1)],
                                    start=False, stop=(k == 2 * NK16 - 1),
                                    skip_group_check=True)
                    # x2 is stored fp16 (ln2h) straight out of the STT --
                    # the adapter's final residual add re-materializes f32.
                    for m in ms:
                        for n2 in range(2):
                            nc.vector.scalar_tensor_tensor(
                                out=ln2h[m][:, TQ * n2:TQ * (n2 + 1)],
                                in0=pts[(m, n2)][:],
                                scalar=1.0 / WS,
                                in1=x1[m][:, TQ * n2:TQ * (n2 + 1)],
                                op0=ALU.mult, op1=ALU.add)
                        nc.sync.dma_start_transpose(ln2T[:, m, :, :], ln2h[m][:])

                mproj_pass((0, 1, 2, 3))

                nc.gpsimd.dma_start(w_ad_sb[:], w_ad[:])
                nc.gpsimd.dma_start(w_au_sb[:], w_au[:])

                # adapter, interleaved per token block: up(m) only needs
                # down-quarter tg=m, so each block drains to DRAM while the
                # next block computes. All fp16; tanh-GELU (same table set
                # as fc; exact-vs-tanh difference is ~5e-4 absolute, washed
                # out by the x0.02 ad_up weights).
                aT = pB.tile([P, 2, TQ], f16, tag="aT", name="aT")
                for m in range(4):
                    for M in range(2):
                        pt = ps.tile([P, TQ], f32, tag="mm", name="mm")
                        for k in range(8):
                            nc.tensor.matmul(
                                pt[:, 0:P], w_ad_sb[:, k, P * M:P * (M + 1)],
                                ln2T[:, m, k, :],
                                start=(k == 0), stop=(k == 7))
                        nc.scalar.activation(
                            aT[:, M, P * m:P * (m + 1)],
                            pt[:, 0:P], AF.Gelu_apprx_tanh,
                            bias=adb_sb[:, M:M + 1])
                    for n2 in range(2):
                        pt = ps.tile([P, TQ], f32, tag="mm", name="mm")
                        for k in range(2):
                            nc.tensor.matmul(
                                pt[:], aT[:, k, P * m:P * (m + 1)],
                                w_au_sb[:, k, TQ * n2:TQ * (n2 + 1)],
                                start=(k == 0), stop=(k == 1))
                        # quarter-wise evict/store so the y DMA overlaps
                        # the next eviction (trims the kernel tail)
                        for q4 in range(2):
                            sl = slice(TQ * n2 + 256 * q4,
                                       TQ * n2 + 256 * (q4 + 1))
                            nc.vector.scalar_tensor_tensor(
                                out=x2[m][:, sl], in0=pt[:, 256 * q4:256 * (q4 + 1)],
                                scalar=1.0, in1=ln2h[m][:, sl],
                                op0=ALU.mult, op1=ALU.add)
                            eng = nc.sync if n2 == 0 else nc.scalar
                            eng.dma_start(y_r[m][:, sl], x2[m][:, sl])

    _split_excess_waits(nc)
    return nc


def _prep_inputs(inputs):
    f16 = ml_dtypes.float16 if hasattr(ml_dtypes, "float16") else np.float16
    f8 = ml_dtypes.float8_e4m3fn
    f32 = np.float32

    def q8(w):
        return np.clip(np.asarray(w, f32) * WS, -240.0, 240.0).astype(f8)

    x = np.ascontiguousarray(np.asarray(inputs["x"], f32))
    vf = np.asarray(inputs["visual_features"], f32)
    # collapsed cross-attention (uniform softmax over identical keys)
    ca_add = ((vf @ np.asarray(inputs["v_w"], f32)
               + np.asarray(inputs["v_b"], f32))
              @ np.asarray(inputs["ca_proj_w"], f32)
              + np.asarray(inputs["ca_proj_b"], f32))        # [B, C]

    # host-side LN1 (pure function of the input x; gain folded into attn_w)
    mu = x.mean(axis=-1, keepdims=True)
    var = np.square(x - mu).mean(axis=-1, keepdims=True)
    h_full = (x - mu) / np.sqrt(var + 1e-5)          # [B, T, C] f32

    # fold LN gains into the consuming weights
    g1 = np.asarray(inputs["ln1_g"], f32)[:, None]
    g2 = np.asarray(inputs["ln2_g"], f32)[:, None]
    attn_w = np.asarray(inputs["attn_w"], f32) * g1
    attn_b = np.asarray(inputs["attn_b"], f32)
    tri = np.triu(np.ones((P, P), f32))          # tri[k, q] = 1 iff k <= q

    def chunks2(w, nch):
        # [C_in, n_out] -> [nch, P, C_in//P, n_out//nch] contiguous per chunk
        cin, cout = w.shape
        return np.ascontiguousarray(
            w.reshape(cin // P, P, nch, cout // nch).transpose(2, 1, 0, 3))

    def kmaj(w):
        # [C_in, n_out] -> [P, C_in//P, n_out]
        cin, cout = w.shape
        return np.ascontiguousarray(
            w.reshape(cin // P, P, cout).transpose(1, 0, 2))

    def kmaj_pair(w):
        # [C_in, n_out] -> [P, C_in//256, 2, n_out]  (DoubleRow k-pairs)
        cin, cout = w.shape
        return np.ascontiguousarray(
            w.reshape(cin // 256, 2, P, cout).transpose(2, 0, 1, 3))

    wq = attn_w[:, :C]
    wk = attn_w[:, C:2 * C]
    wv_ = attn_w[:, 2 * C:]

    def dq(a):
        return np.asarray(a, f32)

    wv_pair = [kmaj_pair(wv_[:, n2 * TQ:(n2 + 1) * TQ]) for n2 in range(2)]
    wv8 = [q8(w) for w in wv_pair]
    # x1024 fp8 residual of the x32 fp8 v-weights (hi/lo correction)
    wv_lo = [np.clip((w - dq(w8) / WS) * (WS * WS), -240.0, 240.0)
             .astype(f8) for w, w8 in zip(wv_pair, wv8)]

    fc_w = np.asarray(inputs["fc_w"], f32) * g2
    # fc chunks: [16, P, 8, 256]; k-pairs 0:J8 fp8, rest fp16, all x32
    fc_ch = chunks2(fc_w, 16)
    w_fc8 = np.ascontiguousarray(
        fc_ch[:, :, 0:J8].reshape(16, P, J8 // 2, 2, 2 * P))
    mp_w = np.asarray(inputs["mproj_w"], f32)
    mp_k = kmaj(mp_w)            # [P, 32, C]
    w_mp8 = np.ascontiguousarray(mp_k[:, 0:K8].reshape(P, K8 // 2, 2, C))

    shared = {
        "w_qkv": np.stack([
            q8(kmaj_pair(np.concatenate(
                [wq[:, hp * P:(hp + 1) * P],
                 wk[:, hp * P:(hp + 1) * P]], axis=1)))
            for hp in range(8)], axis=0),
        "w_v": np.stack(wv8, axis=0),
        "w_v_lo": np.stack(wv_lo, axis=0),
        "w_pj": q8(kmaj_pair(np.asarray(inputs["attn_proj_w"], f32))),
        "w_pj16": kmaj(np.asarray(inputs["attn_proj_w"], f32)).astype(f16),
        "w_fc8": q8(w_fc8),
        "w_fc16": (fc_ch[:, :, J8:] * WS).astype(f16),
        "w_mp8": q8(w_mp8),
        "w_mp16": (mp_k[:, K8:] * WS).astype(f16),
        "w_ad": kmaj(np.asarray(inputs["ad_down_w"], f32)).astype(f16),
        "w_au": kmaj(np.asarray(inputs["ad_up_w"], f32)).astype(f16),
        "bqk_T": np.ascontiguousarray(
            (attn_b[:2 * C] * WS).reshape(16, P).T),
        "fcb_T": np.ascontiguousarray(
            np.asarray(inputs["fc_b"], f32).reshape(FF // P, P).T),
        "adb_T": np.ascontiguousarray(
            np.asarray(inputs["ad_down_b"], f32).reshape(DOWN // P, P).T),
        "tri": tri.astype(f16),
    }
    pj_bias = np.asarray(inputs["attn_proj_b"], f32)[None, :] + ca_add

    in_maps = []
    for c in range(NCORES):
        b, half = c // 2, c % 2
        xq = x[b, TQ * half:TQ * half + TQ]
        m = dict(shared)
        m["x_q"] = np.ascontiguousarray(xq + pj_bias[b][None, :])
        hkv = np.concatenate(
            [h_full[b, 0:TQ], h_full[b, TQ * half:TQ * half + TQ]], axis=0)
        # hT_in[p, a, e, i, f] = h[i*128+f, (2a+e)*128+p]
        hT8 = np.ascontiguousarray(
            np.clip(hkv, -240.0, 240.0)
            .reshape(8, P, 4, 2, P).transpose(4, 2, 3, 0, 1)).astype(f8)
        m["hT_in"] = hT8
        # x32 fp8 residual of tile i=4 (rows TQ*half : TQ*half+128)
        lo4 = (hkv.reshape(8, P, 4, 2, P).transpose(4, 2, 3, 0, 1)[:, :, :, 4, :]
               - dq(hT8[:, :, :, 4, :])) * WS
        m["h_lo4"] = np.clip(lo4, -240.0, 240.0).astype(f8)
        m["log_s"] = np.array([[0.0 if half == 1 else NEG]], f32)
        if "vtag" in _CACHE:
            m["vtag"] = np.zeros((1, _CACHE["vtag"]), f32)
        in_maps.append(m)
    return in_maps


def _bust_stale_neff_cache(nc):
    """The PJRT/neuronxcc compile cache keys on the HLO wrapper module,
    which does NOT change when only this bass program's instructions
    change (same I/O signature) -- a stale NEFF would silently run.
    Wipe the cache whenever the built program's hash differs from the
    marker left by the previous build."""
    import hashlib
    import os
    import shutil
    h = hashlib.md5()
    for fn in nc.m.functions:
        for bb in fn.blocks:
            for ins in bb.instructions:
                h.update(str(ins).encode())
    sig = h.hexdigest()
    cdir = os.path.expanduser("~/.neuron-compile-cache")
    marker = os.path.join(cdir, "bass_kernel_sig.txt")
    try:
        with open(marker) as f:
            if f.read().strip() == sig:
                return
    except OSError:
        pass
    shutil.rmtree(cdir, ignore_errors=True)
    os.makedirs(cdir, exist_ok=True)
    with open(marker, "w") as f:
        f.write(sig)


def _program_hash(nc):
    import hashlib
    h = hashlib.md5()
    for fn in nc.m.functions:
        for bb in fn.blocks:
            for ins in bb.instructions:
                h.update(str(ins).encode())
    return h.hexdigest()


def kernel(**inputs) -> np.ndarray:
    import concourse.mybir as mybir
    from concourse.bass_utils import run_bass_kernel_spmd

    if "nc" not in _CACHE:
        nc = _build_program()
        _bust_stale_neff_cache(nc)
        # Fold the program hash into the HLO signature: several compile
        # caches key on the HLO module, which otherwise does not change
        # when only this bass program's instructions change -- a stale
        # NEFF would silently run. A dummy input whose width encodes the
        # hash forces a distinct module per program version.
        _CACHE["vtag"] = 1 + int(_program_hash(nc)[:4], 16) % 509
        nc.dram_tensor("vtag", [1, _CACHE["vtag"]], mybir.dt.float32,
                       kind="ExternalInput")
        _CACHE["nc"] = nc
    nc = _CACHE["nc"]

    in_maps = _prep_inputs(inputs)
    for m in in_maps:
        m["vtag"] = np.zeros((1, _CACHE["vtag"]), np.float32)
    res = run_bass_kernel_spmd(nc, in_maps, list(range(NCORES)))

    out = np.zeros((B, T, C), np.float32)
    for c in range(NCORES):
        b, half = c // 2, c % 2
        out[b, TQ * half:TQ * half + TQ] = res.results[c]["y"]
    return out
